# revision 7
# baseline (speedup 1.0000x reference)
"""LocalGraphAttention TRN2 kernel: 8-core SPMD (data-parallel B x head-parallel).

Layout strategy (per core c): b = c//2, heads = 4*(c%2) + [0..3].
Everything kept transposed so the softmax key-reduction is a PE matmul:
  xT (D, G) -> QT/KT stacks (128=4h*32, G) -> S^T = K @ Q^T per head
  (4-head packed via tile_position row tiling, contraction d=32),
  exp on ScalarE (PSUM->SBUF bf16, no rowmax needed: |scores|<4),
  multiplicative 0/1 mask on VectorE, P@V + rowsum via col-tiled matmuls
  accumulating y^T (128=4h*32, G) and rowsums in PSUM, normalize with
  reciprocal + k=1 broadcast matmul, out-projection to O^T partial.
Host gathers: out[b] = (OT_even + OT_odd).T + b_out.
"""
import sys
import numpy as np
import ml_dtypes

sys.path.insert(0, "/opt/trn_rl_repo")

from contextlib import ExitStack

import concourse.bass as bass
import concourse.mybir as mybir
import concourse.tile as tile
from concourse import bacc
from concourse.bass_utils import run_bass_kernel_spmd

BF16 = ml_dtypes.bfloat16
G = 2048
D = 256
NH = 8
DH = 32
B = 4
NCORES = 8
SCALE = 1.0 / np.sqrt(np.float32(DH))
KB = G // 128   # 16 key blocks
QG = G // 512   # 4 query groups


def build_nc():
    nc = bacc.Bacc("TRN2", target_bir_lowering=False, debug=False)
    dt = mybir.dt
    xT = nc.declare_dram_parameter("xT", [D, G], dt.bfloat16, isOutput=False)
    Wq = nc.declare_dram_parameter("Wq", [D, 128], dt.bfloat16, isOutput=False)
    Wk = nc.declare_dram_parameter("Wk", [D, 128], dt.bfloat16, isOutput=False)
    Wv = nc.declare_dram_parameter("Wv", [D, 128], dt.bfloat16, isOutput=False)
    bq = nc.declare_dram_parameter("bq", [128, 1], dt.float32, isOutput=False)
    bk = nc.declare_dram_parameter("bk", [128, 1], dt.float32, isOutput=False)
    bv = nc.declare_dram_parameter("bv", [128, 1], dt.float32, isOutput=False)
    M01T = nc.declare_dram_parameter("M01T", [G, G], dt.bfloat16, isOutput=False)
    WoE = nc.declare_dram_parameter("WoE", [128, D], dt.bfloat16, isOutput=False)
    OUT = nc.declare_dram_parameter("out", [D, G], dt.float32, isOutput=True)

    with tile.TileContext(nc) as tc, ExitStack() as ctx:
        singles = ctx.enter_context(tc.tile_pool(name="singles", bufs=1))
        maskp = ctx.enter_context(tc.tile_pool(name="maskp", bufs=KB))
        vp = ctx.enter_context(tc.tile_pool(name="vp", bufs=KB))
        work = ctx.enter_context(tc.tile_pool(name="work", bufs=4))
        psq = ctx.enter_context(tc.tile_pool(name="psq", bufs=3, space="PSUM"))
        ppv = ctx.enter_context(tc.tile_pool(name="ppv", bufs=1, space="PSUM"))

        # ---- resident loads ----
        xt = []
        for kc in range(2):
            t = singles.tile([128, G], dt.bfloat16, tag=f"xt{kc}")
            nc.sync.dma_start(out=t[:], in_=xT[128 * kc:128 * (kc + 1), :])
            xt.append(t)
        wght = {}
        for name, p in (("wq", Wq), ("wk", Wk), ("wv", Wv)):
            for kc in range(2):
                t = singles.tile([128, 128], dt.bfloat16, tag=f"{name}{kc}")
                nc.sync.dma_start(out=t[:], in_=p[128 * kc:128 * (kc + 1), :])
                wght[f"{name}{kc}"] = t
        bq_sb = singles.tile([128, 1], dt.float32, tag="bq")
        nc.sync.dma_start(out=bq_sb[:], in_=bq[:])
        bk_sb = singles.tile([128, 1], dt.float32, tag="bk")
        nc.sync.dma_start(out=bk_sb[:], in_=bk[:])
        bv_sb = singles.tile([128, 1], dt.float32, tag="bv")
        nc.sync.dma_start(out=bv_sb[:], in_=bv[:])
        woe_sb = singles.tile([128, D], dt.bfloat16, tag="woe")
        nc.sync.dma_start(out=woe_sb[:], in_=WoE[:])
        m_sb = []
        for kb in range(KB):
            t = maskp.tile([128, G], dt.bfloat16, tag="mask")
            nc.sync.dma_start(out=t[:], in_=M01T[128 * kb:128 * (kb + 1), :])
            m_sb.append(t)
        ones_sb = singles.tile([128, 1], dt.bfloat16, tag="ones")
        nc.vector.memset(ones_sb[:], 1.0)
        ind_sb = []
        for h in range(4):
            t = singles.tile([1, 128], dt.bfloat16, tag=f"ind{h}")
            nc.vector.memset(t[:], 0.0)
            nc.vector.memset(t[0:1, 32 * h:32 * (h + 1)], 1.0)
            ind_sb.append(t)

        # ---- QKV projections ----
        qt_sb = singles.tile([128, G], dt.bfloat16, tag="qt")
        kt_sb = singles.tile([128, G], dt.bfloat16, tag="kt")
        for dst, wn, b_sb in ((qt_sb, "wq", bq_sb), (kt_sb, "wk", bk_sb)):
            for qg in range(QG):
                ps = psq.tile([128, 1024], dt.float32, tag="sq")
                sl = slice(512 * qg, 512 * (qg + 1))
                nc.tensor.matmul(ps[:, 0:512], wght[wn + "0"][:],
                                 xt[0][:, sl], start=True, stop=False)
                nc.tensor.matmul(ps[:, 0:512], wght[wn + "1"][:],
                                 xt[1][:, sl], start=False, stop=True)
                nc.vector.tensor_scalar_add(dst[:, sl], ps[:, 0:512], b_sb[:])
        v_sb = []
        for kb in range(KB):
            ps = psq.tile([128, 1024], dt.float32, tag="sq")
            sl = slice(128 * kb, 128 * (kb + 1))
            nc.tensor.matmul(ps[:, 0:128], xt[0][:, sl], wght["wv0"][:],
                             start=True, stop=False)
            nc.tensor.matmul(ps[:, 0:128], xt[1][:, sl], wght["wv1"][:],
                             start=False, stop=True)
            t = vp.tile([128, 128], dt.bfloat16, tag="v")
            nc.vector.tensor_copy(t[:], ps[:, 0:128])
            v_sb.append(t)

        # ---- attention ----
        yn_sb = singles.tile([128, G], dt.bfloat16, tag="yn")
        for qg in range(QG):
            qsl = slice(512 * qg, 512 * (qg + 1))
            pv_ps = ppv.tile([128, 512], dt.float32, tag="pv")
            rs_ps = ppv.tile([128, 512], dt.float32, tag="rs")
            for kb in range(KB):
                for pair in range(2):
                    sq = psq.tile([128, 1024], dt.float32, tag="sq")
                    for j in range(2):
                        h = 2 * pair + j
                        hsl = slice(32 * h, 32 * (h + 1))
                        nc.tensor.matmul(
                            sq[:, 512 * j:512 * (j + 1)],
                            kt_sb[hsl, 128 * kb:128 * (kb + 1)],
                            qt_sb[hsl, qsl],
                            start=True, stop=True, tile_position=(32 * h, 0))
                    e = work.tile([128, 1024], dt.bfloat16, tag="e")
                    nc.scalar.activation(e[:], sq[:],
                                         mybir.ActivationFunctionType.Exp,
                                         scale=float(SCALE))
                    em = work.tile([128, 1024], dt.bfloat16, tag="em")
                    for j in range(2):
                        esl = slice(512 * j, 512 * (j + 1))
                        nc.vector.tensor_mul(em[:, esl], e[:, esl],
                                             m_sb[kb][:, qsl])
                    for j in range(2):
                        h = 2 * pair + j
                        esl = slice(512 * j, 512 * (j + 1))
                        nc.tensor.matmul(
                            pv_ps[32 * h:32 * (h + 1), :],
                            v_sb[kb][:, 32 * h:32 * (h + 1)], em[:, esl],
                            start=(kb == 0), stop=(kb == KB - 1),
                            tile_position=(0, 32 * h), skip_group_check=True)
                        nc.tensor.matmul(
                            rs_ps[32 * h:32 * h + 1, :],
                            ones_sb[:], em[:, esl],
                            start=(kb == 0), stop=(kb == KB - 1),
                            tile_position=(0, 32 * h), skip_group_check=True)
            # normalize: yn = pv * (1/rowsum) + bv
            recs = []
            with nc.allow_low_precision("softmax recip bf16"):
                for h in range(4):
                    rec = work.tile([1, 512], dt.bfloat16, tag=f"rec{h}")
                    nc.vector.reciprocal(rec[:], rs_ps[32 * h:32 * h + 1, :])
                    recs.append(rec)
            bc_ps = psq.tile([128, 1024], dt.float32, tag="sq")
            for h in range(4):
                nc.tensor.matmul(bc_ps[:, 0:512], ind_sb[h][:],
                                 recs[h][:], start=(h == 0),
                                 stop=(h == 3), skip_group_check=True)
            bc_sb = work.tile([128, 512], dt.float32, tag="bcs")
            nc.vector.tensor_copy(bc_sb[:], bc_ps[:, 0:512])
            t1 = work.tile([128, 512], dt.float32, tag="t1")
            nc.vector.tensor_mul(t1[:], pv_ps[:], bc_sb[:])
            nc.vector.tensor_scalar_add(yn_sb[:, qsl], t1[:], bv_sb[:])

        # ---- out projection: O^T = WoE.T @ yn ----
        for mt in range(2):
            for qg in range(QG):
                qsl = slice(512 * qg, 512 * (qg + 1))
                ps = psq.tile([128, 1024], dt.float32, tag="sq")
                nc.tensor.matmul(ps[:, 0:512],
                                 woe_sb[:, 128 * mt:128 * (mt + 1)],
                                 yn_sb[:, qsl], start=True, stop=True)
                ot = work.tile([128, 512], dt.float32, tag="ot")
                nc.vector.tensor_copy(ot[:], ps[:, 0:512])
                nc.sync.dma_start(out=OUT[128 * mt:128 * (mt + 1), qsl],
                                  in_=ot[:])
    nc.finalize()
    return nc


_NC_CACHE = None


def kernel(x, allow_mask_bool, W_qkv, b_qkv, W_out, b_out):
    global _NC_CACHE
    x = np.asarray(x, np.float32)
    allow = np.asarray(allow_mask_bool)
    W_qkv = np.asarray(W_qkv, np.float32)
    b_qkv = np.asarray(b_qkv, np.float32)
    W_out = np.asarray(W_out, np.float32)
    b_out = np.asarray(b_out, np.float32)

    M01T = np.ascontiguousarray(allow.T).astype(BF16)
    in_maps = []
    for c in range(NCORES):
        b = c // 2
        hs = [4 * (c % 2) + i for i in range(4)]
        qcols = np.concatenate([np.arange(32 * h, 32 * h + 32) for h in hs])
        m = {
            "xT": np.ascontiguousarray(x[b].T).astype(BF16),
            "Wq": np.ascontiguousarray(W_qkv[:, qcols]).astype(BF16),
            "Wk": np.ascontiguousarray(W_qkv[:, 256 + qcols]).astype(BF16),
            "Wv": np.ascontiguousarray(W_qkv[:, 512 + qcols]).astype(BF16),
            "bq": np.ascontiguousarray(b_qkv[qcols][:, None]),
            "bk": np.ascontiguousarray(b_qkv[256 + qcols][:, None]),
            "bv": np.ascontiguousarray(b_qkv[512 + qcols][:, None]),
            "M01T": M01T,
            "WoE": np.ascontiguousarray(W_out[qcols, :]).astype(BF16),
        }
        in_maps.append(m)

    if _NC_CACHE is None:
        _NC_CACHE = build_nc()
    res = run_bass_kernel_spmd(_NC_CACHE, in_maps, core_ids=list(range(NCORES)))
    out = np.zeros((B, G, D), np.float32)
    for c in range(NCORES):
        out[c // 2] += res.results[c]["out"].T
    out += b_out[None, None, :]
    return out


if __name__ == "__main__":
    rng = np.random.default_rng(0)
    ins = {
        "x": rng.standard_normal((B, G, D), dtype=np.float32),
        "allow_mask_bool": rng.random((G, G)) < 0.5,
        "W_qkv": rng.standard_normal((D, 3 * D), dtype=np.float32) * 0.06,
        "b_qkv": rng.standard_normal(3 * D).astype(np.float32) * 0.06,
        "W_out": rng.standard_normal((D, D), dtype=np.float32) * 0.06,
        "b_out": rng.standard_normal(D).astype(np.float32) * 0.06,
    }
    ins["allow_mask_bool"] |= np.eye(G, dtype=bool)
    out = kernel(**ins)
    print("kernel ran, out shape", out.shape)


# revision 8
# speedup vs baseline: 8584.1299x; 8584.1299x over previous
"""LocalGraphAttention TRN2 kernel: 8-core SPMD (data-parallel B x head-parallel).

Layout strategy (per core c): b = c//2, heads = 4*(c%2) + [0..3].
Everything kept transposed so the softmax key-reduction is a PE matmul:
  xT (D, G) -> QT/KT stacks (128=4h*32, G) -> S^T = K @ Q^T per head
  (4-head packed via tile_position row tiling, contraction d=32),
  exp on ScalarE (PSUM->SBUF bf16, no rowmax needed: |scores|<4),
  multiplicative 0/1 mask on VectorE, P@V + rowsum via col-tiled matmuls
  accumulating y^T (128=4h*32, G) and rowsums in PSUM, normalize with
  reciprocal + k=1 broadcast matmul, out-projection to O^T partial.
Host gathers: out[b] = (OT_even + OT_odd).T + b_out.
"""
import sys
import numpy as np
import ml_dtypes

sys.path.insert(0, "/opt/trn_rl_repo")

from contextlib import ExitStack

import concourse.bass as bass
import concourse.mybir as mybir
import concourse.tile as tile
from concourse import bacc
from concourse.bass_utils import run_bass_kernel_spmd

BF16 = ml_dtypes.bfloat16
G = 2048
D = 256
NH = 8
DH = 32
B = 4
NCORES = 8
SCALE = 1.0 / np.sqrt(np.float32(DH))
KB = G // 128   # 16 key blocks
QG = G // 512   # 4 query groups


def build_nc():
    nc = bacc.Bacc("TRN2", target_bir_lowering=False, debug=False)
    dt = mybir.dt
    xT = nc.declare_dram_parameter("xT", [D, G], dt.bfloat16, isOutput=False)
    Wq = nc.declare_dram_parameter("Wq", [D, 128], dt.bfloat16, isOutput=False)
    Wk = nc.declare_dram_parameter("Wk", [D, 128], dt.bfloat16, isOutput=False)
    Wv = nc.declare_dram_parameter("Wv", [D, 128], dt.bfloat16, isOutput=False)
    bq = nc.declare_dram_parameter("bq", [128, 1], dt.float32, isOutput=False)
    bk = nc.declare_dram_parameter("bk", [128, 1], dt.float32, isOutput=False)
    bv = nc.declare_dram_parameter("bv", [128, 1], dt.float32, isOutput=False)
    M01T = nc.declare_dram_parameter("M01T", [G, G], dt.bfloat16, isOutput=False)
    WoE = nc.declare_dram_parameter("WoE", [128, D], dt.bfloat16, isOutput=False)
    OUT = nc.declare_dram_parameter("out", [D, G], dt.float32, isOutput=True)

    with tile.TileContext(nc) as tc, ExitStack() as ctx:
        singles = ctx.enter_context(tc.tile_pool(name="singles", bufs=1))
        maskp = ctx.enter_context(tc.tile_pool(name="maskp", bufs=KB))
        vp = ctx.enter_context(tc.tile_pool(name="vp", bufs=KB))
        work = ctx.enter_context(tc.tile_pool(name="work", bufs=4))
        psq = ctx.enter_context(tc.tile_pool(name="psq", bufs=3, space="PSUM"))
        ppv = ctx.enter_context(tc.tile_pool(name="ppv", bufs=1, space="PSUM"))

        # ---- resident loads ----
        xt = []
        for kc in range(2):
            t = singles.tile([128, G], dt.bfloat16, tag=f"xt{kc}")
            nc.sync.dma_start(out=t[:], in_=xT[128 * kc:128 * (kc + 1), :])
            xt.append(t)
        wght = {}
        for name, p in (("wq", Wq), ("wk", Wk), ("wv", Wv)):
            for kc in range(2):
                t = singles.tile([128, 128], dt.bfloat16, tag=f"{name}{kc}")
                nc.sync.dma_start(out=t[:], in_=p[128 * kc:128 * (kc + 1), :])
                wght[f"{name}{kc}"] = t
        bq_sb = singles.tile([128, 1], dt.float32, tag="bq")
        nc.sync.dma_start(out=bq_sb[:], in_=bq[:])
        bk_sb = singles.tile([128, 1], dt.float32, tag="bk")
        nc.sync.dma_start(out=bk_sb[:], in_=bk[:])
        bv_sb = singles.tile([128, 1], dt.float32, tag="bv")
        nc.sync.dma_start(out=bv_sb[:], in_=bv[:])
        woe_sb = singles.tile([128, D], dt.bfloat16, tag="woe")
        nc.sync.dma_start(out=woe_sb[:], in_=WoE[:])
        m_sb = []
        for kb in range(KB):
            t = maskp.tile([128, G], dt.bfloat16, tag="mask")
            nc.sync.dma_start(out=t[:], in_=M01T[128 * kb:128 * (kb + 1), :])
            m_sb.append(t)
        ones_sb = singles.tile([128, 1], dt.bfloat16, tag="ones")
        nc.vector.memset(ones_sb[:], 1.0)
        ind_sb = []
        for h in range(4):
            t = singles.tile([1, 128], dt.bfloat16, tag=f"ind{h}")
            nc.vector.memset(t[:], 0.0)
            nc.vector.memset(t[0:1, 32 * h:32 * (h + 1)], 1.0)
            ind_sb.append(t)

        # ---- QKV projections ----
        qt_sb = singles.tile([128, G], dt.bfloat16, tag="qt")
        kt_sb = singles.tile([128, G], dt.bfloat16, tag="kt")
        for dst, wn, b_sb in ((qt_sb, "wq", bq_sb), (kt_sb, "wk", bk_sb)):
            for qg in range(QG):
                ps = psq.tile([128, 1024], dt.float32, tag="sq")
                sl = slice(512 * qg, 512 * (qg + 1))
                nc.tensor.matmul(ps[:, 0:512], wght[wn + "0"][:],
                                 xt[0][:, sl], start=True, stop=False)
                nc.tensor.matmul(ps[:, 0:512], wght[wn + "1"][:],
                                 xt[1][:, sl], start=False, stop=True)
                nc.vector.tensor_scalar_add(dst[:, sl], ps[:, 0:512], b_sb[:])
        v_sb = []
        for kb in range(KB):
            ps = psq.tile([128, 1024], dt.float32, tag="sq")
            sl = slice(128 * kb, 128 * (kb + 1))
            nc.tensor.matmul(ps[:, 0:128], xt[0][:, sl], wght["wv0"][:],
                             start=True, stop=False)
            nc.tensor.matmul(ps[:, 0:128], xt[1][:, sl], wght["wv1"][:],
                             start=False, stop=True)
            t = vp.tile([128, 128], dt.bfloat16, tag="v")
            nc.vector.tensor_copy(t[:], ps[:, 0:128])
            v_sb.append(t)

        # ---- attention ----
        yn_sb = singles.tile([128, G], dt.bfloat16, tag="yn")
        for qg in range(QG):
            qsl = slice(512 * qg, 512 * (qg + 1))
            pv_ps = ppv.tile([128, 512], dt.float32, tag="pv")
            rs_ps = ppv.tile([128, 512], dt.float32, tag="rs")
            for kb in range(KB):
                for pair in range(2):
                    sq = psq.tile([128, 1024], dt.float32, tag="sq")
                    for j in range(2):
                        h = 2 * pair + j
                        hsl = slice(32 * h, 32 * (h + 1))
                        nc.tensor.matmul(
                            sq[:, 512 * j:512 * (j + 1)],
                            kt_sb[hsl, 128 * kb:128 * (kb + 1)],
                            qt_sb[hsl, qsl],
                            start=True, stop=True, tile_position=(32 * h, 0))
                    e = work.tile([128, 1024], dt.bfloat16, tag="e")
                    nc.scalar.activation(e[:], sq[:],
                                         mybir.ActivationFunctionType.Exp,
                                         scale=float(SCALE))
                    em = work.tile([128, 1024], dt.bfloat16, tag="em")
                    for j in range(2):
                        esl = slice(512 * j, 512 * (j + 1))
                        nc.vector.tensor_mul(em[:, esl], e[:, esl],
                                             m_sb[kb][:, qsl])
                    for j in range(2):
                        h = 2 * pair + j
                        esl = slice(512 * j, 512 * (j + 1))
                        nc.tensor.matmul(
                            pv_ps[32 * h:32 * (h + 1), :],
                            v_sb[kb][:, 32 * h:32 * (h + 1)], em[:, esl],
                            start=(kb == 0), stop=(kb == KB - 1),
                            tile_position=(0, 32 * h), skip_group_check=True)
                        nc.tensor.matmul(
                            rs_ps[32 * h:32 * h + 1, :],
                            ones_sb[:], em[:, esl],
                            start=(kb == 0), stop=(kb == KB - 1),
                            tile_position=(0, 32 * h), skip_group_check=True)
            # normalize: yn = pv * (1/rowsum) + bv
            recs = []
            with nc.allow_low_precision("softmax recip bf16"):
                for h in range(4):
                    rec = work.tile([1, 512], dt.bfloat16, tag=f"rec{h}")
                    nc.vector.reciprocal(rec[:], rs_ps[32 * h:32 * h + 1, :])
                    recs.append(rec)
            bc_ps = psq.tile([128, 1024], dt.float32, tag="sq")
            for h in range(4):
                nc.tensor.matmul(bc_ps[:, 0:512], ind_sb[h][:],
                                 recs[h][:], start=(h == 0),
                                 stop=(h == 3), skip_group_check=True)
            bc_sb = work.tile([128, 512], dt.float32, tag="bcs")
            nc.vector.tensor_copy(bc_sb[:], bc_ps[:, 0:512])
            t1 = work.tile([128, 512], dt.float32, tag="t1")
            nc.vector.tensor_mul(t1[:], pv_ps[:], bc_sb[:])
            nc.vector.tensor_scalar_add(yn_sb[:, qsl], t1[:], bv_sb[:])

        # ---- out projection: O^T = WoE.T @ yn ----
        for mt in range(2):
            for qg in range(QG):
                qsl = slice(512 * qg, 512 * (qg + 1))
                ps = psq.tile([128, 1024], dt.float32, tag="sq")
                nc.tensor.matmul(ps[:, 0:512],
                                 woe_sb[:, 128 * mt:128 * (mt + 1)],
                                 yn_sb[:, qsl], start=True, stop=True)
                ot = work.tile([128, 512], dt.float32, tag="ot")
                nc.vector.tensor_copy(ot[:], ps[:, 0:512])
                nc.sync.dma_start(out=OUT[128 * mt:128 * (mt + 1), qsl],
                                  in_=ot[:])
    nc.finalize()
    return nc


_NC_CACHE = None


def kernel(x, allow_mask_bool, W_qkv, b_qkv, W_out, b_out):
    global _NC_CACHE
    x = np.asarray(x, np.float32)
    allow = np.asarray(allow_mask_bool)
    W_qkv = np.asarray(W_qkv, np.float32)
    b_qkv = np.asarray(b_qkv, np.float32)
    W_out = np.asarray(W_out, np.float32)
    b_out = np.asarray(b_out, np.float32)

    M01T = np.ascontiguousarray(allow.T).astype(BF16)
    in_maps = []
    for c in range(NCORES):
        b = c // 2
        hs = [4 * (c % 2) + i for i in range(4)]
        qcols = np.concatenate([np.arange(32 * h, 32 * h + 32) for h in hs])
        m = {
            "xT": np.ascontiguousarray(x[b].T).astype(BF16),
            "Wq": np.ascontiguousarray(W_qkv[:, qcols]).astype(BF16),
            "Wk": np.ascontiguousarray(W_qkv[:, 256 + qcols]).astype(BF16),
            "Wv": np.ascontiguousarray(W_qkv[:, 512 + qcols]).astype(BF16),
            "bq": np.ascontiguousarray(b_qkv[qcols][:, None]),
            "bk": np.ascontiguousarray(b_qkv[256 + qcols][:, None]),
            "bv": np.ascontiguousarray(b_qkv[512 + qcols][:, None]),
            "M01T": M01T,
            "WoE": np.ascontiguousarray(W_out[qcols, :]).astype(BF16),
        }
        in_maps.append(m)

    global LAST_IN_MAPS
    LAST_IN_MAPS = in_maps
    if _NC_CACHE is None:
        _NC_CACHE = build_nc()
    res = run_bass_kernel_spmd(_NC_CACHE, in_maps, core_ids=list(range(NCORES)))
    out = np.zeros((B, G, D), np.float32)
    for c in range(NCORES):
        out[c // 2] += res.results[c]["out"].T
    out += b_out[None, None, :]
    return out


if __name__ == "__main__":
    rng = np.random.default_rng(0)
    ins = {
        "x": rng.standard_normal((B, G, D), dtype=np.float32),
        "allow_mask_bool": rng.random((G, G)) < 0.5,
        "W_qkv": rng.standard_normal((D, 3 * D), dtype=np.float32) * 0.06,
        "b_qkv": rng.standard_normal(3 * D).astype(np.float32) * 0.06,
        "W_out": rng.standard_normal((D, D), dtype=np.float32) * 0.06,
        "b_out": rng.standard_normal(D).astype(np.float32) * 0.06,
    }
    ins["allow_mask_bool"] |= np.eye(G, dtype=bool)
    out = kernel(**ins)
    print("kernel ran, out shape", out.shape)
